# revision 24
# baseline (speedup 1.0000x reference)
"""AWD-LSTM forward kernel for 8 Trainium2 NeuronCores.

Strategy: data-parallel over batch. Each core gets 32 batch lanes chosen
flip-closed (16 from the front of the batch + the 16 mirrored ones from the
back), so the reference's batch-flip `h0n[::-1]` becomes a purely local
swap of the two 16-lane halves.

Input staging: the embedding lookup runs on host and ships as a per-core
bf16 X^T [320, 8192] (emb 300 + casing 7 + pos 12 + ones row); the bulky
weights (wh0 bf16, wi1|wh1 fp8 DoubleRow-paired) ship row-sharded and are
AllGathered on device once per call.

Phase 1 (the sequential recurrence) stays all-bf16: measured pair cost for
[128x128]@[128x32] ldweights+matmul streams is ~28ns in bf16 and fp8 gives
no gain (and mixed dtypes disable FWL and are 2-4x slower).  Phase 1 also
maintains an fp8 mirror of h (x2^6, one extra scalar op per step) stored
to DRAM as H08T for phase 2.

Phase 2 (batched layer-1: no self-recurrence, it reads the batch-flipped
layer-0 state) runs its gate matmuls in fp8 DoubleRow: contraction pairs
of 128-row chunks load as [128,2,M] stationary against [128,2,N] moving,
halving the pair count (measured 129ns vs 158ns per pair at N=512, i.e.
2.4x per unit work).  Weights are scaled 2^8 and h 2^6 into fp8 normal
range; the 2^14 PSUM scale is undone exactly by the gate activations
(func(in*scale + bias), bias unscaled).  The decode matmuls and the cell
element-wise path stay bf16/fp32, so fp8 noise only enters h1 through the
gates (~1% RMS), not the decoded h0 path.

Per core, three phases:
  0: batched input projection PRE0X = wi0_aug @ X^T (bias folded via the
     ones feature row), written to DRAM.
  1: sequential LSTM-0 recurrence over T=256 steps (For_i over 16 slabs
     of 16 steps).  Gate-major layout: pre^T [4096 gate rows -> 32
     chunks of 128, 32 lanes].  bf16 matmuls, fp32 cell state.
  2: layer-1 + decode, For_i over 16 column blocks of 512 (t,lane) pairs.
"""

import contextlib
import os
import sys

for _p in ("/opt/trn_rl_repo", "/root/.axon_site/_ro/trn_rl_repo"):
    if os.path.isdir(_p) and _p not in sys.path:
        sys.path.insert(0, _p)

import ml_dtypes
import numpy as np

import concourse.bass as bass
import concourse.tile as tile
from concourse.masks import make_identity
from concourse import bacc, mybir
from concourse.bass_utils import run_bass_kernel_spmd

F32 = mybir.dt.float32
BF16 = mybir.dt.bfloat16
FP8 = mybir.dt.float8e4
I32 = mybir.dt.int32
AF = mybir.ActivationFunctionType
OP = mybir.AluOpType
DR = mybir.MatmulPerfMode.DoubleRow
BF16NP = ml_dtypes.bfloat16
FP8NP = ml_dtypes.float8_e4m3

T, B, H, E, V, C = 256, 256, 1024, 300, 50000, 13
NCORES = 8
LB = 32                  # local batch lanes per core
NCOL = T * LB            # 8192 (t-major columns)
G4 = 4 * H               # 4096 gate rows
MCH = G4 // 128          # 32 gate chunks
KCH = H // 128           # 8 hidden chunks
KP = KCH // 2            # 4 DoubleRow chunk pairs
SLAB = 16                # steps per phase-1 slab
NSLAB = T // SLAB        # 16
BLK = 512                # phase-2 column block
NBLK = NCOL // BLK       # 16
FEAT = 320               # padded feature rows (300 emb + 7 + 12 + ones)
WSCALE = 256.0           # 2^8 scale on wi1/wh1 fp8
HSCALE = 64.0            # 2^6 scale on fp8 h mirror
P2SCALE = WSCALE * HSCALE


def _local_batch(k):
    front = list(range(16 * k, 16 * k + 16))
    back = list(range(255 - 16 * k, 239 - 16 * k, -1))
    return front + back


def _build(phases=(0, 1, 2), repeat=1, static=False, do_ag=True, p1_free=False):
    # p1_free: TIMING DIAGNOSTIC ONLY -- break the step-to-step dependency
    # (gate matmuls read the carry columns instead of the previous step's h)
    # to measure phase-1's dependency-stall share.  Output is WRONG.
    nc = bacc.Bacc("TRN2", target_bir_lowering=False, debug=False,
                   num_devices=NCORES)

    # ---- per-core DRAM I/O ----
    xT = nc.dram_tensor("xT", [FEAT, NCOL], BF16, kind="ExternalInput")
    wi0T = nc.dram_tensor("wi0T", [FEAT, G4], BF16, kind="ExternalInput")
    # bulky weights arrive as ONE bf16-typed pack, sharded row-wise and
    # AllGathered on device: rows 0:1024 = wh0^T bf16; rows 1024:2048 =
    # the fp8 wi1|wh1 DoubleRow tiles' bytes (host pre-interleaved so tile
    # t is pack row block t; fp8 is recovered via AP bitcast -- an fp8
    # collective/DMA garbles data on this stack, bf16 bytes move cleanly)
    PKR = 2 * H  # 2048 pack rows
    wpsh = nc.dram_tensor("wpsh", [PKR // NCORES, G4], BF16, kind="ExternalInput")
    b1s = nc.dram_tensor("b1s", [2, MCH, 128], F32, kind="ExternalInput")  # bi1; bh1
    decT = nc.dram_tensor("decT", [2 * H, C], BF16, kind="ExternalInput")
    decb = nc.dram_tensor("decb", [C, 1], F32, kind="ExternalInput")
    hcinit = nc.dram_tensor("hcinit", [2, KCH, 128, LB], F32, kind="ExternalInput")
    dec = nc.dram_tensor("dec", [C, NCOL], F32, kind="ExternalOutput")

    # ---- AllGathered weight tensors ----
    if do_ag:
        agin = nc.dram_tensor("agin", [PKR // NCORES, G4], BF16)
        wpS = nc.dram_tensor("wpS", [PKR, G4], BF16, addr_space="Shared")
        wpk = nc.dram_tensor("wpk", [PKR, G4], BF16)
    else:
        wpk = nc.dram_tensor("wpk", [PKR, G4], BF16, kind="ExternalInput")
    wh0T = wpk[0:H, :]
    w8pk = wpk[H:2 * H, :]  # bf16-typed bytes of the 8 fp8 DR tiles

    # ---- scratch DRAM ----
    PRE0X = nc.dram_tensor("PRE0X", [MCH, 128, NCOL + BLK], BF16)  # +pad col block
    H0T = nc.dram_tensor("H0T", [KCH, 128, NCOL], BF16)
    H08T = nc.dram_tensor("H08T", [KCH, 128, NCOL], FP8)  # 2^6 * h0
    C0T = nc.dram_tensor("C0T", [KCH, 128, NCOL], BF16)

    with tile.TileContext(nc) as tc:
      if do_ag:
          nc.sync.dma_start(agin[:], wpsh[:])
          nc.gpsimd.collective_compute(
              "AllGather", mybir.AluOpType.bypass,
              ins=[agin[:]], outs=[wpS[:]],
              replica_groups=[list(range(NCORES))],
          )
          for q in range(4):
              nc.sync.dma_start(wpk[512 * q:512 * (q + 1), :],
                                wpS[512 * q:512 * (q + 1), :])
      with (contextlib.nullcontext(0) if static else tc.For_i(0, repeat, 1)) as _rep:
        # ================= phase 0: PRE0X = wi0_aug @ X^T =================
        if 0 in phases:
         with tc.tile_pool(name="p0sb", bufs=1) as p0, \
             tc.tile_pool(name="p0ps", bufs=4, space="PSUM") as pp0, \
             tc.tile_pool(name="p0st", bufs=2) as pst:

            kszs = [128, 128, 64]
            koff = [0, 128, 256]
            xt = [p0.tile([ksz, NCOL], BF16, tag=f"xt{c}", name=f"xt{c}")
                  for c, ksz in enumerate(kszs)]
            for c in range(3):
                nc.sync.dma_start(xt[c][:], xT[koff[c]:koff[c] + kszs[c], :])

            wi0sb = [p0.tile([ksz, G4], BF16, tag=f"wi0{c}", name=f"wi0{c}")
                     for c, ksz in enumerate(kszs)]
            for c in range(3):
                nc.gpsimd.dma_start(wi0sb[c][:], wi0T[koff[c]:koff[c] + kszs[c], :])

            for m in range(MCH):
                stg = pst.tile([128, NCOL], BF16, tag="stage")
                for n in range(NCOL // 512):
                    ps = pp0.tile([128, 512], F32, tag="ps0")
                    for c in range(3):
                        nc.tensor.matmul(
                            ps[:], wi0sb[c][:, 128 * m:128 * (m + 1)],
                            xt[c][:, 512 * n:512 * (n + 1)],
                            start=(c == 0), stop=(c == 2))
                    if n % 2 == 0:
                        nc.vector.tensor_copy(stg[:, 512 * n:512 * (n + 1)], ps[:])
                    else:
                        nc.scalar.copy(stg[:, 512 * n:512 * (n + 1)], ps[:])
                nc.sync.dma_start(PRE0X[m, :, 0:NCOL], stg[:])

        # ================= phase 1: recurrence =================
        # Half-split software pipeline: hidden chunks split into halves
        # A (0:4) and B (4:8).  The layer-0 gate columns are host-permuted
        # so half X's gates (i|f|o|g, 4 chunks each) occupy one contiguous
        # 512-col PSUM bank.  Per step the PE stream is
        #   foldA, (kA->mA), foldB, (kA->mB), (kB->mA), (kB->mB)
        # so half A's preacts complete two stages early: its elementwise
        # tail (act -> DVE -> tanh -> h) runs while the PE is still doing
        # half B's matmuls, and the next step's (kA->mA) only needs h_A --
        # the PE never sits out a full tail-chain latency per step.
        if 1 in phases:
         with tc.tile_pool(name="p1w", bufs=1) as p1w, \
             tc.tile_pool(name="p1x", bufs=1) as p1x, \
             tc.tile_pool(name="p1p", bufs=2) as p1p, \
             tc.tile_pool(name="p1s", bufs=2) as p1s, \
             tc.tile_pool(name="p1ps", bufs=2, space="PSUM") as p1ps:

            HALF = KCH // 2  # 4
            wh0sb = [p1w.tile([128, G4], BF16, tag=f"wh0{k}", name=f"wh0{k}") for k in range(KCH)]
            for k in range(KCH):
                nc.gpsimd.dma_start(wh0sb[k][:], wh0T[128 * k:128 * (k + 1), :])
            ident = p1w.tile([128, 128], BF16)
            make_identity(nc, ident[:])

            # per-half state: h (bf16), 2^6*h (fp8, phase-2 mirror), c (f32)
            # laid out [128, HALF*(32+SLAB*32)]: col 0:32 carry, 32:544 slab
            CW = 32 + SLAB * 32  # 544
            hh = [p1x.tile([128, HALF * CW], BF16, tag=f"hh{x}", name=f"hh{x}")
                  for x in range(2)]
            h8h = [p1x.tile([128, HALF * CW], FP8, tag=f"h8h{x}", name=f"h8h{x}")
                   for x in range(2)]
            ch = [p1x.tile([128, HALF * CW], F32, tag=f"ch{x}", name=f"ch{x}")
                  for x in range(2)]
            hv = [t[:].rearrange("p (k s) -> p k s", k=HALF) for t in hh]
            h8v = [t[:].rearrange("p (k s) -> p k s", k=HALF) for t in h8h]
            cv = [t[:].rearrange("p (k s) -> p k s", k=HALF) for t in ch]
            for x in range(2):
                for k in range(HALF):
                    nc.gpsimd.dma_start(hv[x][:, k, 0:LB], hcinit[0, HALF * x + k, :, :])
                    nc.sync.dma_start(cv[x][:, k, 0:LB], hcinit[1, HALF * x + k, :, :])

            with (contextlib.nullcontext(0) if static else
                  tc.For_i(0, NSLAB, 1, hint_engines=(mybir.EngineType.PE,))) as it:
                # double-buffered px: next slab's loads prefetch on the sync
                # queue while this slab computes (H0T writes go on gpsimd so
                # they don't block the prefetch in queue order)
                prex = p1p.tile([128, MCH * 512], BF16, tag="prex")
                px3 = prex[:].rearrange("p (m s) -> p m s", m=MCH)
                for m in range(MCH):
                    nc.sync.dma_start(px3[:, m, :], PRE0X[m, :, bass.ts(it, 512)])
                for s in range(SLAB):
                    sc = slice(0, 32) if p1_free else slice(32 * s, 32 * s + 32)
                    # PSUM: one bank per half; the identity fold's start=True
                    # at the bank's first m' clears has_written for the whole
                    # bank, later writes with start=False overwrite where
                    # clear / accumulate where set.
                    pp = [p1ps.tile([128, 16 * LB], F32, tag=f"pp{x}",
                                    name=f"pp{x}") for x in range(2)]

                    def fold(x):
                        for q in range(16):
                            nc.tensor.matmul(
                                pp[x][:, LB * q:LB * (q + 1)], ident[:],
                                px3[:, 16 * x + q, 32 * s:32 * s + 32],
                                start=(q == 0), stop=False,
                                skip_group_check=True)

                    def gates(kx, mx):
                        for q in range(16):
                            mp = 16 * mx + q
                            for k in range(HALF):
                                nc.tensor.matmul(
                                    pp[mx][:, LB * q:LB * (q + 1)],
                                    wh0sb[HALF * kx + k][:, 128 * mp:128 * (mp + 1)],
                                    hv[kx][:, k, sc],
                                    start=False,
                                    stop=(kx == 1 and k == HALF - 1),
                                    skip_group_check=True)

                    fold(0)
                    gates(0, 0)
                    fold(1)
                    gates(0, 1)
                    gates(1, 0)
                    gates(1, 1)

                    for x in range(2):
                        gx = p1s.tile([128, 16 * LB], BF16, tag=f"g{x}",
                                      name=f"g{x}")
                        nc.scalar.activation(gx[:, 0:384], pp[x][:, 0:384], AF.Sigmoid)
                        nc.scalar.activation(gx[:, 384:512], pp[x][:, 384:512], AF.Tanh)
                        t1 = p1s.tile([128, HALF * LB], F32, tag=f"t1{x}",
                                      name=f"t1{x}")
                        nc.vector.tensor_tensor(t1[:], gx[:, 0:128], gx[:, 384:512], op=OP.mult)
                        t13 = t1[:].rearrange("p (k l) -> p k l", k=HALF)
                        f3 = gx[:, 128:256].rearrange("p (k l) -> p k l", k=HALF)
                        o3 = gx[:, 256:384].rearrange("p (k l) -> p k l", k=HALF)
                        cold = cv[x][:, :, sc]
                        cnew = cv[x][:, :, 32 * s + 32:32 * s + 64]
                        nc.vector.tensor_tensor(cnew, f3, cold, op=OP.mult)
                        nc.vector.tensor_tensor(cnew, cnew, t13, op=OP.add)
                        thb = p1s.tile([128, HALF * LB], BF16, tag=f"th{x}",
                                       name=f"th{x}")
                        th3 = thb[:].rearrange("p (k l) -> p k l", k=HALF)
                        nc.scalar.activation(th3, cnew, AF.Tanh)
                        hnew = hv[x][:, :, 32 * s + 32:32 * s + 64]
                        nc.vector.tensor_tensor(hnew, o3, th3, op=OP.mult)
                        nc.scalar.activation(h8v[x][:, :, 32 * s + 32:32 * s + 64],
                                             hnew, AF.Identity, scale=HSCALE)
                # write slab outputs, then carry tail -> head
                for x in range(2):
                    for k in range(HALF):
                        kg = HALF * x + k
                        nc.gpsimd.dma_start(H0T[kg, :, bass.ts(it, 512)], hv[x][:, k, 32:CW])
                        nc.gpsimd.dma_start(H08T[kg, :, bass.ts(it, 512)], h8v[x][:, k, 32:CW])
                        nc.gpsimd.dma_start(C0T[kg, :, bass.ts(it, 512)], cv[x][:, k, 32:CW])
                    nc.vector.tensor_copy(hv[x][:, :, 0:32], hv[x][:, :, CW - 32:CW])
                    nc.vector.tensor_copy(cv[x][:, :, 0:32], cv[x][:, :, CW - 32:CW])

        # ================= phase 2: layer 1 (fp8 DoubleRow) + decode =================
        if 2 in phases:
         with tc.tile_pool(name="p2w", bufs=1) as p2w, \
             tc.tile_pool(name="p2l", bufs=2) as p2l, \
             tc.tile_pool(name="p2b", bufs=1) as p2b, \
             tc.tile_pool(name="p2s", bufs=2) as p2s, \
             tc.tile_pool(name="p2ps", bufs=3, space="PSUM") as p2ps, \
             tc.tile_pool(name="p2pd", bufs=2, space="PSUM") as p2pd:

            # DoubleRow stationary tiles, bf16-typed byte carriers: tile t<4 =
            # wi1 chunk pair t, t>=4 = wh1 pair t-4.  Partition p holds fp8
            # rows (256j+p | 256j+128+p) as packed on host; the matmul AP
            # bitcasts to [128, 2, G4] fp8.
            w1sb = [p2w.tile([128, G4], BF16, tag=f"w1{t}", name=f"w1{t}")
                    for t in range(2 * KP)]
            for t in range(2 * KP):
                nc.gpsimd.dma_start(w1sb[t][:], w8pk[128 * t:128 * (t + 1), :])
            wi1v = [w1sb[j][:].bitcast(FP8).rearrange("p (o g) -> p o g", o=2)
                    for j in range(KP)]
            wh1v = [w1sb[KP + j][:].bitcast(FP8).rearrange("p (o g) -> p o g", o=2)
                    for j in range(KP)]
            decsb = [p2w.tile([128, C], BF16, tag=f"dec{k}", name=f"dec{k}") for k in range(16)]
            for k in range(16):
                nc.gpsimd.dma_start(decsb[k][:], decT[128 * k:128 * (k + 1), :])
            dbias = p2w.tile([C, 1], F32)
            nc.sync.dma_start(dbias[:], decb[:])
            # layer-1 bias, per-partition per-chunk: [128, MCH]
            bs1 = p2w.tile([128, MCH], F32)
            nc.gpsimd.dma_start(bs1[:], b1s[0, :, :].rearrange("m p -> p m"))
            nc.gpsimd.dma_start(bs1[:], b1s[1, :, :].rearrange("m p -> p m"),
                                accum_op=OP.add)

            with (contextlib.nullcontext(0) if static else
                  tc.For_i(0, NBLK, 1, hint_engines=(mybir.EngineType.PE,))) as ib:
                h0b = p2l.tile([128, KCH * BLK], BF16, tag="h0b")
                c0b = p2l.tile([128, KCH * BLK], BF16, tag="c0b")
                h08b = p2l.tile([128, KCH * BLK], FP8, tag="h08b")
                h1b = p2b.tile([128, KCH * BLK], BF16, tag="h1b")
                for k in range(KCH):
                    nc.sync.dma_start(h0b[:, BLK * k:BLK * (k + 1)], H0T[k, :, bass.ts(ib, BLK)])
                    nc.sync.dma_start(c0b[:, BLK * k:BLK * (k + 1)], C0T[k, :, bass.ts(ib, BLK)])
                    nc.sync.dma_start(h08b[:, BLK * k:BLK * (k + 1)], H08T[k, :, bass.ts(ib, BLK)])
                h08fb = p2b.tile([128, KCH * BLK], FP8, tag="h08fb")
                h08b4 = h08b[:].rearrange("p (k t l) -> p k t l", k=KCH, l=32)
                h08f4 = h08fb[:].rearrange("p (k t l) -> p k t l", k=KCH, l=32)
                nc.vector.tensor_copy(h08f4[:, :, :, 0:16], h08b4[:, :, :, 16:32])
                nc.vector.tensor_copy(h08f4[:, :, :, 16:32], h08b4[:, :, :, 0:16])
                # DoubleRow moving views: [128, KP, 2, BLK]
                h08p = h08b[:].rearrange("p (j o n) -> p j o n", j=KP, o=2)
                h08fp = h08fb[:].rearrange("p (j o n) -> p j o n", j=KP, o=2)
                for j in range(KCH):
                    g1 = p2s.tile([128, 4 * BLK], BF16, tag="g1")
                    for gate in range(4):
                        m = gate * KCH + j
                        pm = p2ps.tile([128, BLK], F32, tag="pm")
                        for jp in range(KP):
                            nc.tensor.matmul(
                                pm[:], wi1v[jp][:, :, 128 * m:128 * (m + 1)],
                                h08p[:, jp, :, :],
                                start=(jp == 0), stop=False,
                                perf_mode=DR)
                        for jp in range(KP):
                            nc.tensor.matmul(
                                pm[:], wh1v[jp][:, :, 128 * m:128 * (m + 1)],
                                h08fp[:, jp, :, :],
                                start=False, stop=(jp == KP - 1),
                                perf_mode=DR)
                        nc.scalar.activation(g1[:, BLK * gate:BLK * (gate + 1)], pm[:],
                                             AF.Sigmoid if gate < 3 else AF.Tanh,
                                             bias=bs1[:, m:m + 1],
                                             scale=1.0 / P2SCALE)
                    i_ = g1[:, 0:BLK]
                    f4 = g1[:, BLK:2 * BLK].rearrange("p (t l) -> p t l", l=32)
                    o_ = g1[:, 2 * BLK:3 * BLK]
                    g_ = g1[:, 3 * BLK:4 * BLK]
                    t1 = p2s.tile([128, BLK], F32, tag="t1b")
                    nc.vector.tensor_tensor(t1[:], i_, g_, op=OP.mult)
                    c1 = p2s.tile([128, BLK], F32, tag="c1")
                    c14 = c1[:].rearrange("p (t l) -> p t l", l=32)
                    c0j = c0b[:, BLK * j:BLK * (j + 1)].rearrange("p (t l) -> p t l", l=32)
                    nc.vector.tensor_tensor(c14[:, :, 0:16], f4[:, :, 0:16], c0j[:, :, 16:32], op=OP.mult)
                    nc.vector.tensor_tensor(c14[:, :, 16:32], f4[:, :, 16:32], c0j[:, :, 0:16], op=OP.mult)
                    nc.vector.tensor_tensor(c1[:], c1[:], t1[:], op=OP.add)
                    th = p2s.tile([128, BLK], BF16, tag="thb2")
                    nc.scalar.activation(th[:], c1[:], AF.Tanh)
                    nc.vector.tensor_tensor(h1b[:, BLK * j:BLK * (j + 1)], o_, th[:], op=OP.mult)
                pd = p2pd.tile([C, BLK], F32, tag="pd")
                for k in range(KCH):
                    nc.tensor.matmul(pd[:], decsb[k][:, :], h0b[:, BLK * k:BLK * (k + 1)],
                                     start=(k == 0), stop=False)
                for j in range(KCH):
                    nc.tensor.matmul(pd[:], decsb[KCH + j][:, :], h1b[:, BLK * j:BLK * (j + 1)],
                                     start=False, stop=(j == KCH - 1))
                dsb = p2s.tile([C, BLK], F32, tag="dsb")
                nc.scalar.activation(dsb[:], pd[:], AF.Identity, bias=dbias[:, 0:1])
                nc.sync.dma_start(dec[:, bass.ts(ib, BLK)], dsb[:])

    nc.compile()
    return nc


_CACHE = {}


def _prep_inputs(tokens, casing, pos, emb_table, wi0, bi0, wh0, bh0,
                 wi1, bi1, wh1, bh1, dec_w, dec_b, h_init, c_init):
    tokens = np.asarray(tokens)
    emb16 = np.asarray(emb_table, np.float32).astype(BF16NP)

    # full gathered X [T, B, 320] in bf16: emb | casing | pos | ones
    xfull = np.empty((T, B, FEAT), BF16NP)
    xfull[:, :, 0:E] = emb16[tokens.reshape(-1)].reshape(T, B, E)
    xfull[:, :, E:E + 7] = np.asarray(casing, np.float32).astype(BF16NP)
    xfull[:, :, E + 7:E + 19] = np.asarray(pos, np.float32).astype(BF16NP)
    xfull[:, :, E + 19] = BF16NP(1.0)

    # wi0_aug (bias folded)
    wi0a = np.zeros((FEAT, G4), np.float32)
    wi0a[0:E + 19, :] = np.asarray(wi0, np.float32).T
    wi0a[FEAT - 1, :] = np.asarray(bi0, np.float32) + np.asarray(bh0, np.float32)
    # layer-0 gate-column permutation for the half-split pipeline:
    # new m' = x*16 + g*4 + c  <-  old m = g*8 + x*4 + c
    # (x = hidden half, g = gate i|f|o|g, c = chunk within half)
    perm = np.concatenate(
        [np.arange(128 * (g * 8 + x * 4 + c), 128 * (g * 8 + x * 4 + c) + 128)
         for x in range(2) for g in range(4) for c in range(4)])
    wi0Tv = wi0a[:, perm].astype(BF16NP)
    wh0Tv = np.asarray(wh0, np.float32).T[:, perm].astype(BF16NP)
    # wi1|wh1 scaled into fp8 normal range, interleaved for the DoubleRow
    # tiles (tile t partition p = fp8 rows 256j+p | 256j+128+p), and the
    # bytes viewed as bf16 so collectives/DMA move them untouched
    w8i = (np.asarray(wi1, np.float32).T * WSCALE).astype(FP8NP)
    w8h = (np.asarray(wh1, np.float32).T * WSCALE).astype(FP8NP)
    pk8 = np.empty((H, 2 * G4), np.uint8)
    for t in range(KP):
        pk8[128 * t:128 * (t + 1), 0:G4] = w8i[256 * t:256 * t + 128].view(np.uint8)
        pk8[128 * t:128 * (t + 1), G4:2 * G4] = w8i[256 * t + 128:256 * t + 256].view(np.uint8)
        u = KP + t
        pk8[128 * u:128 * (u + 1), 0:G4] = w8h[256 * t:256 * t + 128].view(np.uint8)
        pk8[128 * u:128 * (u + 1), G4:2 * G4] = w8h[256 * t + 128:256 * t + 256].view(np.uint8)
    wpkv = np.vstack([wh0Tv, pk8.view(BF16NP)])  # [2048, 4096] bf16

    b1sv = np.stack([np.asarray(bi1, np.float32).reshape(MCH, 128),
                     np.asarray(bh1, np.float32).reshape(MCH, 128)])
    decTv = np.ascontiguousarray(np.asarray(dec_w, np.float32).T).astype(BF16NP)
    decbv = np.asarray(dec_b, np.float32).reshape(C, 1)
    h_init = np.asarray(h_init, np.float32)
    c_init = np.asarray(c_init, np.float32)

    in_maps = []
    for k in range(NCORES):
        lb = _local_batch(k)
        xTk = np.ascontiguousarray(
            xfull[:, lb, :].reshape(NCOL, FEAT).T)          # [320, 8192] bf16
        hc = np.stack([
            np.ascontiguousarray(h_init[0][lb, :].T).reshape(KCH, 128, LB),
            np.ascontiguousarray(c_init[0][lb, :].T).reshape(KCH, 128, LB)])
        shp = 2 * H // NCORES
        in_maps.append({
            "xT": xTk,
            "wi0T": wi0Tv,
            "wpsh": wpkv[shp * k:shp * (k + 1)],
            "b1s": b1sv, "decT": decTv, "decb": decbv,
            "hcinit": hc,
        })
    return in_maps


def _unshard(results):
    out = np.empty((T, B, C), np.float32)
    for k in range(NCORES):
        lb = _local_batch(k)
        d = results[k]["dec"]                      # [13, 8192]
        out[:, lb, :] = d.T.reshape(T, LB, C)
    return out.reshape(T * B, C)


def kernel(**inputs):
    if "nc" not in _CACHE:
        _CACHE["nc"] = _build()
    nc = _CACHE["nc"]
    in_maps = _prep_inputs(**inputs)
    res = run_bass_kernel_spmd(nc, in_maps, core_ids=list(range(NCORES)))
    return _unshard(res.results)
